# revision 2
# baseline (speedup 1.0000x reference)
"""Trainium2 Bass kernel for nn_Attention_62362925138174 (v4).

Reference (per batch b, xf = x[b].reshape(C, N), N = H*W = 4096):
    q = Wq @ xf; k = Wk @ xf; v = Wv @ xf
    score[n, m] = q[:, n] . k[:, m]
    P = softmax(score, axis=n)             (per-column softmax)
    att = gamma * (v @ P) + xf

Kernel strategy (8 cores = 4 batches x 2 column-halves of N):
  - score = xf^T (Wq^T Wk) xk via kg = G @ xk, bf16 (full PE column rate).
  - E = exp(score) fp8(e4m3): ScalarE real Exp + VectorE Schraudolph
    bit-trick, balanced by the measured cost models
    Act (FD+352)/1.2ns, DVE (FD+120)/0.96ns; instructions span TWO
    adjacent PSUM ring slots when the 3-slot ring allows (2/3 of pairs)
    to amortize fixed overhead.
  - PV runs fp8 DoubleRow with vaug = [gamma*v^T | ones64]: output rows
    0:64 accumulate gamma*(v E), rows 64:128 accumulate colsum(E)
    broadcast across 64 partitions FOR FREE (PE output rows cost
    nothing) -- this kills the gpsimd partition_broadcast + its ~6us
    IRAM library loads that serialized v3.
  - Tail per chunk: reciprocal_approx_fast on the den rows, then
    mul + residual-add on DVE, emitted early in the next chunk so the
    O PSUM buffer is released before the next chunk's first PV needs
    it. No gpsimd ops anywhere in the kernel.
  - Inputs: 2 big DMAs (sync + scalar HWDGE rings); xkf input dropped,
    the residual reads the bf16 xfp (error budget ~1e-3 << 2e-2).
"""

import numpy as np

import concourse.bass as bass
import concourse.bacc as bacc
import concourse.tile as tile
from concourse import mybir
from concourse.bass_utils import run_bass_kernel_spmd

B, C, H, W = 4, 64, 64, 64
N = H * W            # 4096
MHALF = N // 2       # 2048 columns of score/output per core
NT = N // 128        # 32 row-tiles of the score matrix
NP = NT // 2         # 16 row-tile pairs
N_CORES = 8

F32 = mybir.dt.float32
BF16 = mybir.dt.bfloat16
FP8 = mybir.dt.float8e4
I8 = mybir.dt.int8
NP_FP8 = mybir.dt.np(FP8)
NP_BF16 = mybir.dt.np(BF16)

EXP = mybir.ActivationFunctionType.Exp
MULT = mybir.AluOpType.mult
ADD = mybir.AluOpType.add
DR = mybir.MatmulPerfMode.DoubleRow

# Schraudolph constants for e4m3 bit-pattern exp (DVE f32->int8 convert
# rounds to nearest).
SCH_A = 8.0 / float(np.log(2.0))       # 11.5416
SCH_B = 56.0 - 0.349                   # RMS-optimal Schraudolph shift

_PROGRAM = None

# exp column split: Act takes X_PAIR of 2048 on paired instructions,
# X_SINGLE of 1024 on split (ring-wrap) pairs. Balanced including the
# per-chunk tail DVE work.
X_PAIR = 1104
X_SINGLE = 520
PV_LAG = 5            # pairs of lag between exp and PV emission


def _build_program() -> bass.Bass:
    nc = bacc.Bacc()

    xfp_d = nc.declare_dram_parameter("xfp", [128, N], BF16, isOutput=False)
    gt16_d = nc.declare_dram_parameter("gt16", [128, 128], BF16, isOutput=False)
    wv16_d = nc.declare_dram_parameter("wv16", [128, C], BF16, isOutput=False)
    out_d = nc.declare_dram_parameter("out", [C, MHALF], F32, isOutput=True)

    from concourse.hw_specs import get_activation_tables

    act_sets = list(get_activation_tables(nc.m.arch))
    nle_id = act_sets.index("natural_log_exp_and_others")

    from contextlib import ExitStack

    with ExitStack() as stack:
        tc = stack.enter_context(tile.TileContext(nc))
        sing = stack.enter_context(tc.tile_pool(name="sing", bufs=1))
        apool = stack.enter_context(tc.tile_pool(name="apool", bufs=2))
        psS = stack.enter_context(tc.tile_pool(name="psS", bufs=1, space="PSUM"))
        psO = stack.enter_context(tc.tile_pool(name="psO", bufs=1, space="PSUM"))

        nc.scalar.add_instruction(
            mybir.InstLoadActFuncSet(
                name=nc.get_next_instruction_name(),
                act_func_set_id=nle_id,
                ins=[],
                outs=[],
            )
        )

        # ---- input DMAs: sync ring carries gt16 -> xfp[0:1024] -> wv16 ->
        # xfp[1024:2048]; scalar ring (Act queue is idle this early)
        # carries xfp[2048:4096] in one big transfer. ----
        gt16_sb = sing.tile([128, 128], BF16, name="gt16_sb")
        nc.sync.dma_start(out=gt16_sb, in_=gt16_d[:, :])
        xfp_sb = sing.tile([128, N], BF16, name="xfp_sb")
        nc.sync.dma_start(out=xfp_sb[:, 0:1024], in_=xfp_d[:, 0:1024])
        wv16_sb = sing.tile([128, C], BF16, name="wv16_sb")
        nc.sync.dma_start(out=wv16_sb, in_=wv16_d[:, :])
        nc.sync.dma_start(out=xfp_sb[:, 1024:2048], in_=xfp_d[:, 1024:2048])
        nc.scalar.dma_start(out=xfp_sb[:, 2048:4096], in_=xfp_d[:, 2048:4096])

        # ---- persistent SBUF tiles ----
        kg_sb = sing.tile([128, MHALF], BF16, name="kg_sb")
        # vaug[n, 0:64] = (gamma Wv xf)^T, vaug[n, 64:128] = 1.0 so PV's
        # DoubleRow output rows 64:128 all accumulate colsum(E).
        vaug_sb = sing.tile([128, NT, 128], FP8, name="vaug_sb")
        nc.vector.memset(vaug_sb[:, :, 64:128], 1.0)
        E_sb = sing.tile([128, NT, 1024], FP8, name="E_sb")
        E_flat = E_sb.rearrange("p a b -> p (a b)")

        # S ring: one PSUM tile, 3 slots of [128, 1024] (banks 0-5);
        # slice-level deps give ring semantics. Slot pairs (0,1) and
        # (1,2) are contiguous -> single wide exp instructions.
        S = psS.tile([128, 3, 1024], F32, name="S_ring")
        S_flat = S.rearrange("p a b -> p (a b)")
        O_ps = psO.tile([128, 1024], F32, name="O_ps")

        # ---- kg = G @ xk [128, MHALF] bf16 (rows 64+ zero via padded
        # gt16). Uses S ring slots as staging before the main loop. ----
        for h in range(2):
            kslot = S[:, h, :]
            for cc in range(2):
                lo = h * 1024 + cc * 512
                nc.tensor.matmul(
                    kslot[:, cc * 512 : (cc + 1) * 512],
                    lhsT=gt16_sb,
                    rhs=xfp_sb[:, lo : lo + 512],
                    start=True,
                    stop=True,
                )
            lo = h * 1024
            nc.scalar.copy(out=kg_sb[:, lo : lo + 512], in_=kslot[:, 0:512])
            nc.vector.tensor_copy(
                out=kg_sb[:, lo + 512 : lo + 1024], in_=kslot[:, 512:1024]
            )

        # ---- vaug v-part: vt = xfp_tile^T @ wv16 in batches of 16 tiles
        # through S slot 2 then slot 0 (slot 0's kg read is done by the
        # time the second batch's matmuls land). Also serves as the PE
        # HAM warm-up burst. ----
        for vv in range(2):
            vslot = S[:, 2 - 2 * vv, :]
            for i in range(16):
                t = vv * 16 + i
                nc.tensor.matmul(
                    vslot[:, i * 64 : (i + 1) * 64],
                    lhsT=xfp_sb[:, t * 128 : (t + 1) * 128],
                    rhs=wv16_sb,
                    start=True,
                    stop=True,
                )
            vtv = vslot.rearrange("p (i u) -> p i u", u=64)
            nc.scalar.copy(
                out=vaug_sb[:, vv * 16 : vv * 16 + 8, 0:64], in_=vtv[:, 0:8, :]
            )
            nc.vector.tensor_copy(
                out=vaug_sb[:, vv * 16 + 8 : vv * 16 + 16, 0:64], in_=vtv[:, 8:16, :]
            )

        OUT_Q = [nc.sync, nc.sync, nc.sync, nc.scalar]

        def tail_steps(ch, final=False):
            """Per-chunk tail: den rows 64:128 of O are the broadcast
            colsum. rcp -> mul -> add on DVE (+1 Act copy), then DMA.
            O is released after the den copy + mul (steps 1-2)."""
            den_sb = apool.tile([C, 1024], F32, tag="den", name="den_sb")
            rcpb = apool.tile([C, 1024], F32, tag="rcpb", name="rcpb")
            tmp = apool.tile([C, 1024], F32, tag="tmp", name="tmp")
            att = apool.tile([C, 1024], F32, tag="att", name="att")
            ocols = slice(ch * 1024, (ch + 1) * 1024)
            # stage den to partition-0 SBUF (rcp_approx_fast mis-reads
            # partition-offset inputs) on Act; O's den half released.
            yield lambda: nc.scalar.copy(out=den_sb, in_=O_ps[64:128, :])
            # mul first releases O's value half early; rcp next.
            yield lambda: nc.vector.reciprocal_approx_fast(out=rcpb, in_=den_sb)
            yield lambda: nc.vector.tensor_mul(tmp, O_ps[0:C, :], rcpb)
            if final:
                yield lambda: (
                    nc.vector.tensor_add(att[:, 0:512], tmp[:, 0:512],
                                         xfp_sb[0:C, ch * 1024 : ch * 1024 + 512]),
                    OUT_Q[ch * 2].dma_start(
                        out=out_d[:, ch * 1024 : ch * 1024 + 512],
                        in_=att[:, 0:512]),
                    nc.vector.tensor_add(att[:, 512:1024], tmp[:, 512:1024],
                                         xfp_sb[0:C, ch * 1024 + 512 : ch * 1024 + 1024]),
                    OUT_Q[ch * 2 + 1].dma_start(
                        out=out_d[:, ch * 1024 + 512 : ch * 1024 + 1024],
                        in_=att[:, 512:1024]),
                )
            else:
                yield lambda: nc.vector.tensor_add(att, tmp, xfp_sb[0:C, ocols])
                yield lambda: OUT_Q[ch * 2].dma_start(out=out_d[:, ocols], in_=att)

        def emit_exp(p, ch):
            """exp of tile pair p (tiles 2p, 2p+1) from S ring into E
            slots. Paired single instructions when ring slots are
            adjacent ascending, else two per-tile instructions."""
            t0 = 2 * p
            s0, s1 = t0 % 3, (t0 + 1) % 3
            if s1 == s0 + 1:
                lo = s0 * 1024
                elo = t0 * 1024
                nc.scalar.activation(
                    out=E_flat[:, elo : elo + X_PAIR],
                    in_=S_flat[:, lo : lo + X_PAIR],
                    func=EXP,
                )
                nc.vector.tensor_scalar(
                    out=E_flat.bitcast(I8)[:, elo + X_PAIR : elo + 2048],
                    in0=S_flat[:, lo + X_PAIR : lo + 2048],
                    scalar1=SCH_A, scalar2=SCH_B, op0=MULT, op1=ADD,
                )
            else:
                for (t, s) in ((t0, s0), (t0 + 1, s1)):
                    nc.scalar.activation(
                        out=E_sb[:, t, 0:X_SINGLE],
                        in_=S[:, s, 0:X_SINGLE],
                        func=EXP,
                    )
                    nc.vector.tensor_scalar(
                        out=E_sb.bitcast(I8)[:, t, X_SINGLE:1024],
                        in0=S[:, s, X_SINGLE:1024],
                        scalar1=SCH_A, scalar2=SCH_B, op0=MULT, op1=ADD,
                    )

        def emit_pv(j, ch):
            vpair = vaug_sb[:, 2 * j : 2 * j + 2, :]
            for cc in range(2):
                nc.tensor.matmul(
                    O_ps[:, cc * 512 : (cc + 1) * 512],
                    lhsT=vpair,
                    rhs=E_sb[:, 2 * j : 2 * j + 2, cc * 512 : (cc + 1) * 512],
                    start=(j == 0),
                    stop=(j == NP - 1),
                    perf_mode=DR,
                )

        prev_tail = None
        for ch in range(2):
            for p in range(NP):
                for t in (2 * p, 2 * p + 1):
                    slot = S[:, t % 3, :]
                    lhsT_t = xfp_sb[:, t * 128 : (t + 1) * 128]
                    for cc in range(2):
                        nc.tensor.matmul(
                            slot[:, cc * 512 : (cc + 1) * 512],
                            lhsT=lhsT_t,
                            rhs=kg_sb[:, ch * 1024 + cc * 512 : ch * 1024 + (cc + 1) * 512],
                            start=True,
                            stop=True,
                        )
                emit_exp(p, ch)
                # one lagged tail step of the previous chunk per pair
                if prev_tail is not None and p >= 1:
                    step = next(prev_tail, None)
                    if step is not None:
                        step()
                    else:
                        prev_tail = None
                if p >= PV_LAG:
                    emit_pv(p - PV_LAG, ch)
            for j in range(NP - PV_LAG, NP):
                emit_pv(j, ch)
            prev_tail = tail_steps(ch, final=(ch == 1))

        for step in prev_tail:
            step()

    nc.finalize()
    return nc


def get_program() -> bass.Bass:
    global _PROGRAM
    if _PROGRAM is None:
        _PROGRAM = _build_program()
    return _PROGRAM


def make_in_maps(x, Wq, Wk, Wv, gamma):
    """Host-side prep: reshape/slice/rotate, dtype casts, zero-padding, and
    weight-only algebra (G = Wq^T Wk folded; gamma folded into Wv)."""
    x = np.ascontiguousarray(np.asarray(x, dtype=np.float32))
    Wq = np.asarray(Wq, dtype=np.float32)
    Wk = np.asarray(Wk, dtype=np.float32)
    Wv = np.asarray(Wv, dtype=np.float32)
    gamma = float(np.asarray(gamma, dtype=np.float32).reshape(()))

    gt16 = np.zeros((128, 128), dtype=NP_BF16)
    gt16[:C, :C] = (Wk.T @ Wq).astype(NP_BF16)      # lhsT for kg = G @ xk
    wv16 = np.zeros((128, C), dtype=NP_BF16)
    wv16[:C, :] = (gamma * Wv.T).astype(NP_BF16)

    in_maps = []
    for core in range(N_CORES):
        b, h = divmod(core, 2)
        xf = x[b].reshape(C, N)
        xk = xf[:, h * MHALF : (h + 1) * MHALF]
        xo = xf[:, (1 - h) * MHALF : (2 - h) * MHALF]
        # rotate so this core's m-half sits at columns 0:MHALF
        xrot = np.concatenate([xk, xo], axis=1)
        xfp = np.zeros((128, N), dtype=NP_BF16)
        xfp[:C] = xrot.astype(NP_BF16)
        in_maps.append(
            {
                "xfp": xfp,
                "gt16": gt16,
                "wv16": wv16,
            }
        )
    return in_maps


def gather(results):
    out = np.empty((B, C, N), dtype=np.float32)
    for core in range(N_CORES):
        b, h = divmod(core, 2)
        out[b][:, h * MHALF : (h + 1) * MHALF] = results[core]["out"]
    return out.reshape(B, C, H, W)


def run(inputs, **spmd_kwargs):
    nc = get_program()
    in_maps = make_in_maps(
        inputs["x"], inputs["Wq"], inputs["Wk"], inputs["Wv"], inputs["gamma"]
    )
    res = run_bass_kernel_spmd(nc, in_maps, core_ids=list(range(N_CORES)), **spmd_kwargs)
    return gather(res.results), res


def kernel(x, Wq, Wk, Wv, gamma):
    out, _ = run({"x": x, "Wq": Wq, "Wk": Wk, "Wv": Wv, "gamma": gamma})
    return out
